# revision 1
# baseline (speedup 1.0000x reference)
"""NaiveFourierKANLayer GNN message passing on 8 Trainium2 NeuronCores.

Strategy:
  m_node[n, j] = sum_{i,k} cos(k x[n,i]) Wc[j,i,k] + sin(k x[n,i]) Ws[j,i,k]
  h[d, j]     = sum_{e: dst[e]=d} m_node[src[e], j]  (+ bias)

- Host: range-reduce the 16 (trig, k) argument planes into [-pi, pi) fp16
  (Sin activation spline is only valid there), bucket edges by
  (dst-block-of-128, src<32768) into a padded slot schedule shared by all
  cores (dst-range edge sharding: core c owns dst in [c*6272, (c+1)*6272)).
- Phase 1 (node-sharded): each core computes m_node for its 6272 nodes:
  ACT Sin -> bf16 Fourier features, PE matmul vs coeffs -> PSUM fp32,
  written to a bf16 [6272, 128] slice (64 msg + 64 zero pad for 256B rows).
- AllGather the slices -> full m_node [50176, 128] bf16 on every core.
- Phase 2 (edge-sharded by dst range): bulk dma_gather of m_node rows per
  edge slot (two passes: src<32768 and src>=32768, int16 idx limit), build
  one-hot(dst_rel) matrices with a DVE is_equal, and scatter-reduce with
  the tensor engine: psum[dst, j] += onehot^T @ m_edge, per 128-dst block.
- Output h slices are concatenated on host; bias added on host.
"""
import math
import numpy as np

N_NODES = 50000
N_EDGES = 800000
IN_F = 32
OUT_F = 64
GRID = 8
N_CORES = 8
NSH = 6272                    # nodes per core (49 * 128)
NPAD = NSH * N_CORES          # 50176
NBLK = NSH // 128             # 49 dst blocks per core
SUPER = 4                     # dst blocks per gather super-group
MROW = 128                    # m_node row width (64 msg + 64 pad) -> 256B bf16

_COMPILED = {}
DEBUG_PHASE = 0
LOOP_N = 1
SKIP_GATHER = False
SKIP_MM = False  # 0=full, 1=phase1+allgather only (h=0), 2=phase2 only (mn as input)


# ---------------------------------------------------------------- host prep

def _host_planes(x):
    """[512, NPAD] fp16 of range-reduced sin args: row 128c + 32g + i."""
    xT = np.zeros((IN_F, NPAD), np.float32)
    xT[:, :N_NODES] = x.T
    R = np.empty((512, NPAD), np.float16)
    for c in range(4):
        phase = 0.5 * np.pi if c < 2 else 0.0
        for g in range(4):
            k = 4 * (c % 2) + g + 1
            arg = k * xT + (phase + np.pi)
            red = np.mod(arg, 2 * np.pi) - np.pi
            R[128 * c + 32 * g: 128 * c + 32 * (g + 1), :] = red.astype(np.float16)
    return R


def _host_coeffs(fouriercoeffs):
    """[512, 64] bf16-as-uint16 view prep: row 128c + 32g + i -> fc[t, j, i, k-1]."""
    W = np.empty((512, OUT_F), np.float32)
    for c in range(4):
        t = 0 if c < 2 else 1
        for g in range(4):
            k = 4 * (c % 2) + g + 1
            # fc[t, :, :, k-1] is [OUT_F, IN_F] -> rows i, cols j
            W[128 * c + 32 * g: 128 * c + 32 * (g + 1), :] = fouriercoeffs[t, :, :, k - 1].T
    return _to_bf16(W)


def _to_bf16(a):
    """fp32 -> bf16 numpy array (ml_dtypes)."""
    import ml_dtypes
    return a.astype(np.float32).astype(ml_dtypes.bfloat16)


def _host_edges(src, dst):
    """Bucket edges into the padded slot schedule.

    Returns (schedule, idx16 [8,128,TOT/16], dstrel_bf16 [8,128,TOTG]).
    schedule: dict with Lg[49], Hg[49], supers list.
    """
    src = np.asarray(src, np.int64)
    dst = np.asarray(dst, np.int64)
    core = dst // NSH
    dst_local = dst - core * NSH
    blk = dst_local >> 7
    rel = dst_local & 127
    high = (src >= 32768).astype(np.int64)
    key = (core * NBLK + blk) * 2 + high
    nkey = N_CORES * NBLK * 2
    cnt = np.bincount(key, minlength=nkey)
    cnt_cbh = cnt.reshape(N_CORES, NBLK, 2)
    Lg = np.maximum(np.ceil(cnt_cbh[:, :, 0].max(axis=0) / 128).astype(np.int64), 0)
    Hg = np.maximum(np.ceil(cnt_cbh[:, :, 1].max(axis=0) / 128).astype(np.int64), 0)

    # slot layout per core: per super: [low slots of blocks | high slots]
    n_sup = math.ceil(NBLK / SUPER)
    base_low = np.zeros(NBLK, np.int64)
    base_high = np.zeros(NBLK, np.int64)
    supers = []
    pos = 0
    for s in range(n_sup):
        bs = list(range(SUPER * s, min(SUPER * (s + 1), NBLK)))
        sup = {"blocks": bs, "slot0": pos}
        for b in bs:
            base_low[b] = pos
            pos += Lg[b] * 128
        sup["low_end"] = pos
        for b in bs:
            base_high[b] = pos
            pos += Hg[b] * 128
        sup["end"] = pos
        supers.append(sup)
    TOT = pos
    TOTG = TOT // 128

    order = np.argsort(key, kind="stable")
    cnt_flat = cnt
    seg_start = np.concatenate([[0], np.cumsum(cnt_flat)[:-1]])
    key_s = key[order]
    rank_s = np.arange(N_EDGES, dtype=np.int64) - seg_start[key_s]
    blk_s = blk[order]
    high_s = high[order]
    core_s = core[order]
    src_s = src[order]
    rel_s = rel[order]
    slot_s = np.where(high_s == 0, base_low[blk_s], base_high[blk_s]) + rank_s

    idxval = (src_s - 32768 * high_s).astype(np.int16)
    idx16 = np.zeros((N_CORES, 16, TOT // 16), np.int16)
    idx16[core_s, slot_s % 16, slot_s // 16] = idxval
    idx16 = np.tile(idx16, (1, 8, 1))  # replicate to 128 partitions

    dstrel = np.full((N_CORES, 128, TOTG), 1000.0, np.float32)
    dstrel[core_s, slot_s % 128, slot_s // 128] = rel_s.astype(np.float32)
    dstrel16 = _to_bf16(dstrel)

    sched = {
        "Lg": tuple(int(v) for v in Lg),
        "Hg": tuple(int(v) for v in Hg),
        "supers": supers,
        "base_low": base_low,
        "base_high": base_high,
        "TOT": TOT,
        "TOTG": TOTG,
    }
    return sched, idx16, dstrel16


# ---------------------------------------------------------------- device program

def _build_program(sched, loop_n=1):
    import contextlib
    import concourse.bacc as bacc
    import concourse.mybir as mybir
    import concourse.tile as tile

    Lg, Hg = sched["Lg"], sched["Hg"]
    TOT, TOTG = sched["TOT"], sched["TOTG"]

    nc = bacc.Bacc("TRN2", target_bir_lowering=False, debug=False,
                   num_devices=N_CORES)
    bf16 = mybir.dt.bfloat16
    fp16 = mybir.dt.float16
    fp32 = mybir.dt.float32
    i16 = mybir.dt.int16

    r2_in = nc.dram_tensor("r2", [512, NSH], fp16, kind="ExternalInput").ap()
    wp_in = nc.dram_tensor("wp", [512, OUT_F], bf16, kind="ExternalInput").ap()
    idx_in = nc.dram_tensor("idx16", [128, TOT // 16], i16, kind="ExternalInput").ap()
    dr_in = nc.dram_tensor("dstrel", [128, TOTG], bf16, kind="ExternalInput").ap()
    iota_in = nc.dram_tensor("iota", [128, 128], bf16, kind="ExternalInput").ap()
    h_out = nc.dram_tensor("h", [NSH, OUT_F], fp32, kind="ExternalOutput").ap()

    if DEBUG_PHASE == 2:
        mn_full = nc.dram_tensor("mn_in", [NPAD, MROW], bf16, kind="ExternalInput").ap()
        mn_slice = None
    else:
        mn_slice = nc.dram_tensor("mn_slice", [NSH, MROW], bf16).ap()
        mn_full = nc.dram_tensor("mn_full", [NPAD, MROW], bf16, addr_space="Shared").ap()

    with tile.TileContext(nc) as tc:
        with (
            tc.tile_pool(name="p1", bufs=2) as p1,
            tc.tile_pool(name="const", bufs=1) as constp,
            tc.tile_pool(name="psum", bufs=4, space="PSUM") as psp,
        ):
            run_p1 = DEBUG_PHASE in (0, 1, 3)
            run_p2 = DEBUG_PHASE in (0, 2)
            wp_t = constp.tile([128, 4, OUT_F], bf16)
            nc.sync.dma_start(out=wp_t[:], in_=wp_in.rearrange("(c p) j -> p c j", p=128))
            iota_t = constp.tile([128, 128], bf16)
            nc.sync.dma_start(out=iota_t[:], in_=iota_in[:])
            dr_t = constp.tile([128, TOTG], bf16)
            nc.sync.dma_start(out=dr_t[:], in_=dr_in[:])
            idx_t = constp.tile([128, TOT // 16], i16)
            nc.sync.dma_start(out=idx_t[:], in_=idx_in[:])
            h_sb = constp.tile([128, NBLK, OUT_F], fp32)

            def emit_phase1():
                tiles = [(s, min(1792, NSH - s)) for s in range(0, NSH, 1792)]
                for (t0, tn) in (tiles if run_p1 else []):
                    Fs = []
                    for c in range(4):
                        xr = p1.tile([128, tn], fp16, tag="xr")
                        nc.sync.dma_start(out=xr[:], in_=r2_in[128 * c:128 * (c + 1), t0:t0 + tn])
                        Fc = p1.tile([128, tn], bf16, tag=f"F{c}")
                        nc.scalar.activation(Fc[:], xr[:], mybir.ActivationFunctionType.Sin)
                        Fs.append(Fc)
                    stag = p1.tile([128, tn // 128, MROW], bf16, tag="stag")
                    nc.scalar.memzero(stag[:])
                    for sub in range(tn // 128):
                        ps = psp.tile([128, OUT_F], fp32, tag="ps1")
                        for c in range(4):
                            nc.tensor.matmul(
                                out=ps[:],
                                lhsT=Fs[c][:, sub * 128:(sub + 1) * 128],
                                rhs=wp_t[:, c, :],
                                start=(c == 0), stop=(c == 3),
                            )
                        nc.vector.tensor_copy(out=stag[:, sub, 0:OUT_F], in_=ps[:])
                    nc.sync.dma_start(
                        out=mn_slice[t0:t0 + tn, :].rearrange("(s p) d -> p s d", p=128),
                        in_=stag[:],
                    )

            def emit_cc():
                if run_p1 and DEBUG_PHASE not in (2, 3):
                    nc.gpsimd.collective_compute(
                        "AllGather", mybir.AluOpType.bypass,
                        replica_groups=[list(range(N_CORES))],
                        ins=[mn_slice[:]], outs=[mn_full[:]],
                    )

            def emit_phase2():
                if not run_p2:
                    nc.scalar.memzero(h_sb[:])
                for sup in (sched["supers"] if run_p2 else []):
                    bs = sup["blocks"]
                    slot0, low_end, end = sup["slot0"], sup["low_end"], sup["end"]
                    gsup = (end - slot0) // 128
                    if gsup == 0:
                        for b in bs:
                            nc.scalar.memzero(h_sb[:, b, :])
                        continue
                    g0 = slot0 // 128
                    me = p1.tile([128, gsup, MROW], bf16, tag="me")
                    n_low = low_end - slot0
                    n_high = end - low_end
                    if SKIP_GATHER:
                        n_low = n_high = 0
                    if n_low > 0:
                        nc.gpsimd.dma_gather(
                            out_ap=me[:, 0:n_low // 128, :],
                            in_ap=mn_full[:],
                            idxs_ap=idx_t[:, slot0 // 16: low_end // 16],
                            num_idxs=n_low, num_idxs_reg=n_low, elem_size=MROW,
                            single_packet=False,
                        )
                    if n_high > 0:
                        nc.gpsimd.dma_gather(
                            out_ap=me[:, n_low // 128: gsup, :],
                            in_ap=mn_full[32768:, :],
                            idxs_ap=idx_t[:, low_end // 16: end // 16],
                            num_idxs=n_high, num_idxs_reg=n_high, elem_size=MROW,
                            single_packet=False,
                        )
                    if SKIP_MM:
                        continue
                    oh = p1.tile([128, gsup, 128], bf16, tag="oh")
                    nc.vector.tensor_tensor(
                        out=oh[:],
                        in0=dr_t[:, g0:g0 + gsup].unsqueeze(2).to_broadcast([128, gsup, 128]),
                        in1=iota_t[:].unsqueeze(1).to_broadcast([128, gsup, 128]),
                        op=mybir.AluOpType.is_equal,
                    )
                    for b in bs:
                        cols = (
                            [(sched["base_low"][b] - slot0) // 128 + j for j in range(Lg[b])]
                            + [(sched["base_high"][b] - slot0) // 128 + j for j in range(Hg[b])]
                        )
                        if not cols:
                            nc.scalar.memzero(h_sb[:, b, :])
                            continue
                        ps = psp.tile([128, OUT_F], fp32, tag="ps2")
                        for gi, col in enumerate(cols):
                            nc.tensor.matmul(
                                out=ps[:],
                                lhsT=oh[:, col, :],
                                rhs=me[:, col, 0:OUT_F],
                                start=(gi == 0), stop=(gi == len(cols) - 1),
                            )
                        nc.vector.tensor_copy(out=h_sb[:, b, :], in_=ps[:])

            if loop_n == 1:
                emit_phase1()
                emit_cc()
                emit_phase2()
            else:
                # timing mode: collective once (on whatever mn_slice holds),
                # then loop phase1+phase2 without the collective.
                emit_cc()
                with tc.For_i(0, loop_n, 1):
                    emit_phase1()
                    emit_phase2()

            nc.sync.dma_start(
                out=h_out[:].rearrange("(b p) j -> p b j", p=128),
                in_=h_sb[:],
            )

    nc.compile()
    return nc


def _build_cc_probe(nreps):
    """Program issuing `nreps` back-to-back AllGathers (for collective timing)."""
    import concourse.bacc as bacc
    import concourse.mybir as mybir
    import concourse.tile as tile

    nc = bacc.Bacc("TRN2", target_bir_lowering=False, debug=False,
                   num_devices=N_CORES)
    bf16 = mybir.dt.bfloat16
    dummy_in = nc.dram_tensor("d", [128, 16], mybir.dt.float32, kind="ExternalInput").ap()
    dummy_out = nc.dram_tensor("o", [128, 16], mybir.dt.float32, kind="ExternalOutput").ap()
    mn_slice = nc.dram_tensor("mn_slice", [NSH, MROW], bf16).ap()
    mn_full = nc.dram_tensor("mn_full", [NPAD, MROW], bf16, addr_space="Shared").ap()
    with tile.TileContext(nc) as tc:
        with tc.tile_pool(name="s", bufs=1) as pool:
            t = pool.tile([128, 16], mybir.dt.float32)
            nc.sync.dma_start(out=t[:], in_=dummy_in[:])
            for _ in range(nreps):
                nc.gpsimd.collective_compute(
                    "AllGather", mybir.AluOpType.bypass,
                    replica_groups=[list(range(N_CORES))],
                    ins=[mn_slice[:]], outs=[mn_full[:]],
                )
            nc.sync.dma_start(out=dummy_out[:], in_=t[:])
    nc.compile()
    return nc


# ---------------------------------------------------------------- runner

def _make_runner(nc):
    """Cached-jit SPMD runner (replicates bass2jax.run_bass_via_pjrt)."""
    import jax
    import jax.numpy as jnp
    import numpy as _np
    from jax.sharding import Mesh, PartitionSpec
    from jax.experimental.shard_map import shard_map
    import concourse.mybir as mybir
    from concourse import bass2jax

    bass2jax.install_neuronx_cc_hook()

    partition_name = nc.partition_id_tensor.name if nc.partition_id_tensor else None
    in_names, out_names, out_avals = [], [], []
    for alloc in nc.m.functions[0].allocations:
        if not isinstance(alloc, mybir.MemoryLocationSet):
            continue
        name = alloc.memorylocations[0].name
        if alloc.kind == "ExternalInput":
            if name != partition_name:
                in_names.append(name)
        elif alloc.kind == "ExternalOutput":
            shape = tuple(alloc.tensor_shape)
            dtype = mybir.dt.np(alloc.dtype)
            out_names.append(name)
            out_avals.append(jax.core.ShapedArray(shape, dtype))
    n_params = len(in_names)
    all_in_names = in_names + out_names
    if partition_name is not None:
        all_in_names = all_in_names + [partition_name]

    def _body(*args):
        operands = list(args)
        if partition_name is not None:
            operands.append(bass2jax.partition_id_tensor())
        outs = bass2jax._bass_exec_p.bind(
            *operands,
            out_avals=tuple(out_avals),
            in_names=tuple(all_in_names),
            out_names=tuple(out_names),
            lowering_input_output_aliases=(),
            sim_require_finite=False,
            sim_require_nnan=False,
            nc=nc,
        )
        return tuple(outs)

    devices = jax.devices()[:N_CORES]
    mesh = Mesh(_np.asarray(devices), ("core",))
    in_specs = (PartitionSpec("core"),) * (n_params + len(out_names))
    out_specs = (PartitionSpec("core"),) * len(out_names)
    sharded = jax.jit(shard_map(_body, mesh=mesh, in_specs=in_specs,
                                out_specs=out_specs, check_rep=False))

    def make_multi(nreps):
        def _multi(*args):
            outs = None
            for _ in range(nreps):
                outs = _body(*args)
            return outs
        return jax.jit(shard_map(_multi, mesh=mesh, in_specs=in_specs,
                                 out_specs=out_specs, check_rep=False))

    def put(in_maps):
        concat = [
            _np.concatenate([_np.asarray(in_maps[c][n]) for c in range(N_CORES)], axis=0)
            for n in in_names
        ]
        concat += [
            _np.zeros((N_CORES * a.shape[0], *a.shape[1:]), a.dtype) for a in out_avals
        ]
        return [jax.device_put(a) for a in concat]

    def dispatch(dev_inputs):
        outs = sharded(*dev_inputs)
        for o in outs:
            o.block_until_ready()
        return outs

    def fetch(outs):
        return {
            name: _np.asarray(outs[i]).reshape(N_CORES, *out_avals[i].shape)
            for i, name in enumerate(out_names)
        }

    def run(dev_inputs):
        return fetch(dispatch(dev_inputs))

    run.dispatch = dispatch
    run.fetch = fetch
    run.make_multi = make_multi
    return put, run


# ---------------------------------------------------------------- entry

def prepare(x, src, dst, fouriercoeffs, bias):
    """Build inputs + compiled program. Returns (runner_put, runner_run, in_maps, bias)."""
    R2 = _host_planes(np.asarray(x, np.float32))
    Wp = _host_coeffs(np.asarray(fouriercoeffs, np.float32))
    sched, idx16, dstrel16 = _host_edges(src, dst)
    iota = _to_bf16(np.tile(np.arange(128, dtype=np.float32)[None, :], (128, 1)))

    key = (sched["Lg"], sched["Hg"], LOOP_N)
    if key not in _COMPILED:
        nc = _build_program(sched, loop_n=LOOP_N)
        _COMPILED[key] = (nc, _make_runner(nc))
    nc, (put, run) = _COMPILED[key]

    in_maps = []
    for c in range(N_CORES):
        in_maps.append({
            "r2": R2[:, c * NSH:(c + 1) * NSH].copy(),
            "wp": Wp,
            "idx16": idx16[c],
            "dstrel": dstrel16[c],
            "iota": iota,
        })
    return put, run, in_maps


def kernel(x, src, dst, fouriercoeffs, bias):
    put, run, in_maps = prepare(x, src, dst, fouriercoeffs, bias)
    dev = put(in_maps)
    outs = run(dev)
    h = outs["h"].reshape(NPAD, OUT_F)[:N_NODES]
    return (h + np.asarray(bias, np.float32)[None, :]).astype(np.float32)



# revision 4
# speedup vs baseline: 1.6615x; 1.6615x over previous
"""NaiveFourierKANLayer GNN message passing on 8 Trainium2 NeuronCores.

Strategy:
  m_node[n, j] = sum_{i,k} cos(k x[n,i]) Wc[j,i,k] + sin(k x[n,i]) Ws[j,i,k]
  h[d, j]     = sum_{e: dst[e]=d} m_node[src[e], j]  (+ bias)

- Host: range-reduce the 16 (trig, k) argument planes into [-pi, pi) fp16
  (Sin activation spline is only valid there), bucket edges by
  (dst-block-of-128, src<32768) into a padded slot schedule shared by all
  cores (dst-range edge sharding: core c owns dst in [c*6272, (c+1)*6272)).
- Phase 1 (node-sharded): each core computes m_node for its 6272 nodes:
  ACT Sin -> bf16 Fourier features, PE matmul vs coeffs -> PSUM fp32,
  written to a bf16 [6272, 128] slice (64 msg + 64 zero pad for 256B rows).
- AllGather the slices -> full m_node [50176, 128] bf16 on every core.
- Phase 2 (edge-sharded by dst range): bulk dma_gather of m_node rows per
  edge slot (two passes: src<32768 and src>=32768, int16 idx limit), build
  one-hot(dst_rel) matrices with a DVE is_equal, and scatter-reduce with
  the tensor engine: psum[dst, j] += onehot^T @ m_edge, per 128-dst block.
- Output h slices are concatenated on host; bias added on host.
"""
import math
import numpy as np

N_NODES = 50000
N_EDGES = 800000
IN_F = 32
OUT_F = 64
GRID = 8
N_CORES = 8
NSH = 6272                    # nodes per core (49 * 128)
NPAD = NSH * N_CORES          # 50176
NBLK = NSH // 128             # 49 dst blocks per core
SUPER = 4                     # dst blocks per gather super-group
MROW = 128                    # m_node row width (64 msg + 64 pad) -> 256B bf16

_COMPILED = {}
DEBUG_PHASE = 0
LOOP_N = 1
SKIP_GATHER = False
SKIP_MM = False  # 0=full, 1=phase1+allgather only (h=0), 2=phase2 only (mn as input)


# ---------------------------------------------------------------- host prep

def _host_planes(x):
    """[512, NPAD] fp16 of range-reduced sin args: row 128c + 32g + i."""
    xT = np.zeros((IN_F, NPAD), np.float32)
    xT[:, :N_NODES] = x.T
    R = np.empty((512, NPAD), np.float16)
    for c in range(4):
        phase = 0.5 * np.pi if c < 2 else 0.0
        for g in range(4):
            k = 4 * (c % 2) + g + 1
            arg = k * xT + (phase + np.pi)
            red = np.mod(arg, 2 * np.pi) - np.pi
            R[128 * c + 32 * g: 128 * c + 32 * (g + 1), :] = red.astype(np.float16)
    return R


def _host_coeffs(fouriercoeffs):
    """[512, 64] bf16-as-uint16 view prep: row 128c + 32g + i -> fc[t, j, i, k-1]."""
    W = np.empty((512, OUT_F), np.float32)
    for c in range(4):
        t = 0 if c < 2 else 1
        for g in range(4):
            k = 4 * (c % 2) + g + 1
            # fc[t, :, :, k-1] is [OUT_F, IN_F] -> rows i, cols j
            W[128 * c + 32 * g: 128 * c + 32 * (g + 1), :] = fouriercoeffs[t, :, :, k - 1].T
    return _to_bf16(W)


def _to_bf16(a):
    """fp32 -> bf16 numpy array (ml_dtypes)."""
    import ml_dtypes
    return a.astype(np.float32).astype(ml_dtypes.bfloat16)


def _host_edges(src, dst):
    """Bucket edges into the padded slot schedule.

    Returns (schedule, idx16 [8,128,TOT/16], dstrel_bf16 [8,128,TOTG]).
    schedule: dict with Lg[49], Hg[49], supers list.
    """
    src = np.asarray(src, np.int64)
    dst = np.asarray(dst, np.int64)
    core = dst // NSH
    dst_local = dst - core * NSH
    blk = dst_local >> 7
    rel = dst_local & 127
    high = (src >= 32768).astype(np.int64)
    key = (core * NBLK + blk) * 2 + high
    nkey = N_CORES * NBLK * 2
    cnt = np.bincount(key, minlength=nkey)
    cnt_cbh = cnt.reshape(N_CORES, NBLK, 2)
    Lg = np.maximum(np.ceil(cnt_cbh[:, :, 0].max(axis=0) / 128).astype(np.int64), 0)
    Hg = np.maximum(np.ceil(cnt_cbh[:, :, 1].max(axis=0) / 128).astype(np.int64), 0)

    # slot layout per core: per super: [low slots of blocks | high slots]
    n_sup = math.ceil(NBLK / SUPER)
    base_low = np.zeros(NBLK, np.int64)
    base_high = np.zeros(NBLK, np.int64)
    supers = []
    pos = 0
    for s in range(n_sup):
        bs = list(range(SUPER * s, min(SUPER * (s + 1), NBLK)))
        sup = {"blocks": bs, "slot0": pos}
        for b in bs:
            base_low[b] = pos
            pos += Lg[b] * 128
        sup["low_end"] = pos
        for b in bs:
            base_high[b] = pos
            pos += Hg[b] * 128
        sup["end"] = pos
        supers.append(sup)
    TOT = pos
    TOTG = TOT // 128

    order = np.argsort(key, kind="stable")
    cnt_flat = cnt
    seg_start = np.concatenate([[0], np.cumsum(cnt_flat)[:-1]])
    key_s = key[order]
    rank_s = np.arange(N_EDGES, dtype=np.int64) - seg_start[key_s]
    blk_s = blk[order]
    high_s = high[order]
    core_s = core[order]
    src_s = src[order]
    rel_s = rel[order]
    slot_s = np.where(high_s == 0, base_low[blk_s], base_high[blk_s]) + rank_s

    idxval = (src_s - 32768 * high_s).astype(np.int16)
    idx16 = np.zeros((N_CORES, 16, TOT // 16), np.int16)
    idx16[core_s, slot_s % 16, slot_s // 16] = idxval
    idx16 = np.tile(idx16, (1, 8, 1))  # replicate to 128 partitions

    dstrel = np.full((N_CORES, 128, TOTG), 1000.0, np.float32)
    dstrel[core_s, slot_s % 128, slot_s // 128] = rel_s.astype(np.float32)
    dstrel16 = _to_bf16(dstrel)

    sched = {
        "Lg": tuple(int(v) for v in Lg),
        "Hg": tuple(int(v) for v in Hg),
        "supers": supers,
        "base_low": base_low,
        "base_high": base_high,
        "TOT": TOT,
        "TOTG": TOTG,
    }
    return sched, idx16, dstrel16


# ---------------------------------------------------------------- device program

def _build_program(sched, loop_n=1):
    import contextlib
    import concourse.bacc as bacc
    import concourse.mybir as mybir
    import concourse.tile as tile

    Lg, Hg = sched["Lg"], sched["Hg"]
    TOT, TOTG = sched["TOT"], sched["TOTG"]

    nc = bacc.Bacc("TRN2", target_bir_lowering=False, debug=False,
                   num_devices=N_CORES, num_swdge_queues=4)
    bf16 = mybir.dt.bfloat16
    fp16 = mybir.dt.float16
    fp32 = mybir.dt.float32
    i16 = mybir.dt.int16

    r2_in = nc.dram_tensor("r2", [512, NSH], fp16, kind="ExternalInput").ap()
    wp_in = nc.dram_tensor("wp", [512, OUT_F], bf16, kind="ExternalInput").ap()
    idx_in = nc.dram_tensor("idx16", [128, TOT // 16], i16, kind="ExternalInput").ap()
    dr_in = nc.dram_tensor("dstrel", [128, TOTG], bf16, kind="ExternalInput").ap()
    iota_in = nc.dram_tensor("iota", [128, 128], bf16, kind="ExternalInput").ap()
    h_out = nc.dram_tensor("h", [NSH, OUT_F], fp32, kind="ExternalOutput").ap()

    if DEBUG_PHASE == 2:
        mn_full = nc.dram_tensor("mn_in", [NPAD, MROW], bf16, kind="ExternalInput").ap()
        mn_slice = None
    else:
        mn_slice = nc.dram_tensor("mn_slice", [NSH, MROW], bf16).ap()
        mn_full = nc.dram_tensor("mn_full", [NPAD, MROW], bf16, addr_space="Shared").ap()

    with tile.TileContext(nc) as tc:
        with (
            tc.tile_pool(name="p1", bufs=2) as p1,
            tc.tile_pool(name="const", bufs=1) as constp,
            tc.tile_pool(name="psum", bufs=4, space="PSUM") as psp,
        ):
            run_p1 = DEBUG_PHASE in (0, 1, 3)
            run_p2 = DEBUG_PHASE in (0, 2)
            wp_t = constp.tile([128, 4, OUT_F], bf16)
            nc.sync.dma_start(out=wp_t[:], in_=wp_in.rearrange("(c p) j -> p c j", p=128))
            iota_t = constp.tile([128, 128], bf16)
            nc.sync.dma_start(out=iota_t[:], in_=iota_in[:])
            dr_t = constp.tile([128, TOTG], bf16)
            nc.sync.dma_start(out=dr_t[:], in_=dr_in[:])
            idx_t = constp.tile([128, TOT // 16], i16)
            nc.sync.dma_start(out=idx_t[:], in_=idx_in[:])
            h_sb = constp.tile([128, NBLK, OUT_F], fp32)

            def emit_phase1():
                tiles = [(s, min(1792, NSH - s)) for s in range(0, NSH, 1792)]
                for (t0, tn) in (tiles if run_p1 else []):
                    Fs = []
                    for c in range(4):
                        xr = p1.tile([128, tn], fp16, tag="xr")
                        nc.sync.dma_start(out=xr[:], in_=r2_in[128 * c:128 * (c + 1), t0:t0 + tn])
                        Fc = p1.tile([128, tn], bf16, tag=f"F{c}")
                        nc.scalar.activation(Fc[:], xr[:], mybir.ActivationFunctionType.Sin)
                        Fs.append(Fc)
                    stag = p1.tile([128, tn // 128, MROW], bf16, tag="stag")
                    nc.scalar.memzero(stag[:])
                    for sub in range(tn // 128):
                        ps = psp.tile([128, OUT_F], fp32, tag="ps1")
                        for c in range(4):
                            nc.tensor.matmul(
                                out=ps[:],
                                lhsT=Fs[c][:, sub * 128:(sub + 1) * 128],
                                rhs=wp_t[:, c, :],
                                start=(c == 0), stop=(c == 3),
                            )
                        nc.vector.tensor_copy(out=stag[:, sub, 0:OUT_F], in_=ps[:])
                    nc.sync.dma_start(
                        out=mn_slice[t0:t0 + tn, :].rearrange("(s p) d -> p s d", p=128),
                        in_=stag[:],
                    )

            def emit_cc():
                if run_p1 and DEBUG_PHASE not in (2, 3):
                    nc.gpsimd.collective_compute(
                        "AllGather", mybir.AluOpType.bypass,
                        replica_groups=[list(range(N_CORES))],
                        ins=[mn_slice[:]], outs=[mn_full[:]],
                    )

            def emit_phase2():
                if not run_p2:
                    nc.scalar.memzero(h_sb[:])
                qn = [0]  # round-robin SWDGE queue so desc-gen uses all 4 Q7 pairs

                def next_q():
                    q = qn[0]
                    qn[0] = (q + 1) % 4
                    return q
                for sup in (sched["supers"] if run_p2 else []):
                    bs = sup["blocks"]
                    slot0, low_end, end = sup["slot0"], sup["low_end"], sup["end"]
                    gsup = (end - slot0) // 128
                    if gsup == 0:
                        for b in bs:
                            nc.scalar.memzero(h_sb[:, b, :])
                        continue
                    g0 = slot0 // 128
                    me = p1.tile([128, gsup, MROW], bf16, tag="me")
                    n_low = low_end - slot0
                    n_high = end - low_end
                    if SKIP_GATHER:
                        n_low = n_high = 0
                    if n_low > 0:
                        nc.gpsimd.dma_gather(
                            out_ap=me[:, 0:n_low // 128, :],
                            in_ap=mn_full[:],
                            idxs_ap=idx_t[:, slot0 // 16: low_end // 16],
                            num_idxs=n_low, num_idxs_reg=n_low, elem_size=MROW,
                            single_packet=False, queue_num=next_q(),
                        )
                    if n_high > 0:
                        nc.gpsimd.dma_gather(
                            out_ap=me[:, n_low // 128: gsup, :],
                            in_ap=mn_full[32768:, :],
                            idxs_ap=idx_t[:, low_end // 16: end // 16],
                            num_idxs=n_high, num_idxs_reg=n_high, elem_size=MROW,
                            single_packet=False, queue_num=next_q(),
                        )
                    if SKIP_MM:
                        continue
                    oh = p1.tile([128, gsup, 128], bf16, tag="oh")
                    nc.vector.tensor_tensor(
                        out=oh[:],
                        in0=dr_t[:, g0:g0 + gsup].unsqueeze(2).to_broadcast([128, gsup, 128]),
                        in1=iota_t[:].unsqueeze(1).to_broadcast([128, gsup, 128]),
                        op=mybir.AluOpType.is_equal,
                    )
                    for b in bs:
                        cols = (
                            [(sched["base_low"][b] - slot0) // 128 + j for j in range(Lg[b])]
                            + [(sched["base_high"][b] - slot0) // 128 + j for j in range(Hg[b])]
                        )
                        if not cols:
                            nc.scalar.memzero(h_sb[:, b, :])
                            continue
                        ps = psp.tile([128, OUT_F], fp32, tag="ps2")
                        for gi, col in enumerate(cols):
                            nc.tensor.matmul(
                                out=ps[:],
                                lhsT=oh[:, col, :],
                                rhs=me[:, col, 0:OUT_F],
                                start=(gi == 0), stop=(gi == len(cols) - 1),
                            )
                        nc.vector.tensor_copy(out=h_sb[:, b, :], in_=ps[:])

            if loop_n == 1:
                emit_phase1()
                emit_cc()
                emit_phase2()
            else:
                # timing mode: collective once (on whatever mn_slice holds),
                # then loop phase1+phase2 without the collective.
                emit_cc()
                with tc.For_i(0, loop_n, 1):
                    emit_phase1()
                    emit_phase2()

            nc.sync.dma_start(
                out=h_out[:].rearrange("(b p) j -> p b j", p=128),
                in_=h_sb[:],
            )

    nc.compile()
    return nc


def _build_cc_probe(nreps):
    """Program issuing `nreps` back-to-back AllGathers (for collective timing)."""
    import concourse.bacc as bacc
    import concourse.mybir as mybir
    import concourse.tile as tile

    nc = bacc.Bacc("TRN2", target_bir_lowering=False, debug=False,
                   num_devices=N_CORES)
    bf16 = mybir.dt.bfloat16
    dummy_in = nc.dram_tensor("d", [128, 16], mybir.dt.float32, kind="ExternalInput").ap()
    dummy_out = nc.dram_tensor("o", [128, 16], mybir.dt.float32, kind="ExternalOutput").ap()
    mn_slice = nc.dram_tensor("mn_slice", [NSH, MROW], bf16).ap()
    mn_full = nc.dram_tensor("mn_full", [NPAD, MROW], bf16, addr_space="Shared").ap()
    with tile.TileContext(nc) as tc:
        with tc.tile_pool(name="s", bufs=1) as pool:
            t = pool.tile([128, 16], mybir.dt.float32)
            nc.sync.dma_start(out=t[:], in_=dummy_in[:])
            for _ in range(nreps):
                nc.gpsimd.collective_compute(
                    "AllGather", mybir.AluOpType.bypass,
                    replica_groups=[list(range(N_CORES))],
                    ins=[mn_slice[:]], outs=[mn_full[:]],
                )
            nc.sync.dma_start(out=dummy_out[:], in_=t[:])
    nc.compile()
    return nc


# ---------------------------------------------------------------- runner

def _make_runner(nc):
    """Cached-jit SPMD runner (replicates bass2jax.run_bass_via_pjrt)."""
    import jax
    import jax.numpy as jnp
    import numpy as _np
    from jax.sharding import Mesh, PartitionSpec
    from jax.experimental.shard_map import shard_map
    import concourse.mybir as mybir
    from concourse import bass2jax

    bass2jax.install_neuronx_cc_hook()

    partition_name = nc.partition_id_tensor.name if nc.partition_id_tensor else None
    in_names, out_names, out_avals = [], [], []
    for alloc in nc.m.functions[0].allocations:
        if not isinstance(alloc, mybir.MemoryLocationSet):
            continue
        name = alloc.memorylocations[0].name
        if alloc.kind == "ExternalInput":
            if name != partition_name:
                in_names.append(name)
        elif alloc.kind == "ExternalOutput":
            shape = tuple(alloc.tensor_shape)
            dtype = mybir.dt.np(alloc.dtype)
            out_names.append(name)
            out_avals.append(jax.core.ShapedArray(shape, dtype))
    n_params = len(in_names)
    all_in_names = in_names + out_names
    if partition_name is not None:
        all_in_names = all_in_names + [partition_name]

    def _body(*args):
        operands = list(args)
        if partition_name is not None:
            operands.append(bass2jax.partition_id_tensor())
        outs = bass2jax._bass_exec_p.bind(
            *operands,
            out_avals=tuple(out_avals),
            in_names=tuple(all_in_names),
            out_names=tuple(out_names),
            lowering_input_output_aliases=(),
            sim_require_finite=False,
            sim_require_nnan=False,
            nc=nc,
        )
        return tuple(outs)

    devices = jax.devices()[:N_CORES]
    mesh = Mesh(_np.asarray(devices), ("core",))
    in_specs = (PartitionSpec("core"),) * (n_params + len(out_names))
    out_specs = (PartitionSpec("core"),) * len(out_names)
    sharded = jax.jit(shard_map(_body, mesh=mesh, in_specs=in_specs,
                                out_specs=out_specs, check_rep=False))

    def make_multi(nreps):
        def _multi(*args):
            outs = None
            for _ in range(nreps):
                outs = _body(*args)
            return outs
        return jax.jit(shard_map(_multi, mesh=mesh, in_specs=in_specs,
                                 out_specs=out_specs, check_rep=False))

    def put(in_maps):
        concat = [
            _np.concatenate([_np.asarray(in_maps[c][n]) for c in range(N_CORES)], axis=0)
            for n in in_names
        ]
        concat += [
            _np.zeros((N_CORES * a.shape[0], *a.shape[1:]), a.dtype) for a in out_avals
        ]
        return [jax.device_put(a) for a in concat]

    def dispatch(dev_inputs):
        outs = sharded(*dev_inputs)
        for o in outs:
            o.block_until_ready()
        return outs

    def fetch(outs):
        return {
            name: _np.asarray(outs[i]).reshape(N_CORES, *out_avals[i].shape)
            for i, name in enumerate(out_names)
        }

    def run(dev_inputs):
        return fetch(dispatch(dev_inputs))

    run.dispatch = dispatch
    run.fetch = fetch
    run.make_multi = make_multi
    return put, run


# ---------------------------------------------------------------- entry

def prepare(x, src, dst, fouriercoeffs, bias):
    """Build inputs + compiled program. Returns (runner_put, runner_run, in_maps, bias)."""
    R2 = _host_planes(np.asarray(x, np.float32))
    Wp = _host_coeffs(np.asarray(fouriercoeffs, np.float32))
    sched, idx16, dstrel16 = _host_edges(src, dst)
    iota = _to_bf16(np.tile(np.arange(128, dtype=np.float32)[None, :], (128, 1)))

    key = (sched["Lg"], sched["Hg"], LOOP_N)
    if key not in _COMPILED:
        nc = _build_program(sched, loop_n=LOOP_N)
        _COMPILED[key] = (nc, _make_runner(nc))
    nc, (put, run) = _COMPILED[key]

    in_maps = []
    for c in range(N_CORES):
        in_maps.append({
            "r2": R2[:, c * NSH:(c + 1) * NSH].copy(),
            "wp": Wp,
            "idx16": idx16[c],
            "dstrel": dstrel16[c],
            "iota": iota,
        })
    return put, run, in_maps


def kernel(x, src, dst, fouriercoeffs, bias):
    put, run, in_maps = prepare(x, src, dst, fouriercoeffs, bias)
    dev = put(in_maps)
    outs = run(dev)
    h = outs["h"].reshape(NPAD, OUT_F)[:N_NODES]
    return (h + np.asarray(bias, np.float32)[None, :]).astype(np.float32)



# revision 8
# speedup vs baseline: 2.0245x; 1.2185x over previous
"""NaiveFourierKANLayer GNN message passing on 8 Trainium2 NeuronCores.

Strategy:
  m_node[n, j] = sum_{i,k} cos(k x[n,i]) Wc[j,i,k] + sin(k x[n,i]) Ws[j,i,k]
  h[d, j]     = sum_{e: dst[e]=d} m_node[src[e], j]  (+ bias)

- Host: range-reduce the 16 (trig, k) argument planes into [-pi, pi) fp16
  (Sin activation spline is only valid there), bucket edges by
  (dst-block-of-128, src<32768) into a padded slot schedule shared by all
  cores (dst-range edge sharding: core c owns dst in [c*6272, (c+1)*6272)).
- Phase 1 (node-sharded): each core computes m_node for its 6272 nodes:
  ACT Sin -> bf16 Fourier features, PE matmul vs coeffs -> PSUM fp32,
  written to a bf16 [6272, 128] slice (64 msg + 64 zero pad for 256B rows).
- AllGather the slices -> full m_node [50176, 128] bf16 on every core.
- Phase 2 (edge-sharded by dst range): bulk dma_gather of m_node rows per
  edge slot (two passes: src<32768 and src>=32768, int16 idx limit), build
  one-hot(dst_rel) matrices with a DVE is_equal, and scatter-reduce with
  the tensor engine: psum[dst, j] += onehot^T @ m_edge, per 128-dst block.
- Output h slices are concatenated on host; bias added on host.
"""
import math
import numpy as np

N_NODES = 50000
N_EDGES = 800000
IN_F = 32
OUT_F = 64
GRID = 8
N_CORES = 8
NSH = 6272                    # nodes per core (49 * 128)
NPAD = NSH * N_CORES          # 50176
NBLK = NSH // 128             # 49 dst blocks per core
SUPER = 4                     # dst blocks per gather super-group
MROW = 128                    # m_node row width (64 msg + 64 pad) -> 256B bf16

_COMPILED = {}
DEBUG_PHASE = 0
LOOP_N = 1
SKIP_GATHER = False
SKIP_MM = False  # 0=full, 1=phase1+allgather only (h=0), 2=phase2 only (mn as input)


# ---------------------------------------------------------------- host prep

def _host_planes(x):
    """[512, NPAD] fp16 of range-reduced sin args: row 128c + 32g + i."""
    xT = np.zeros((IN_F, NPAD), np.float32)
    xT[:, :N_NODES] = x.T
    R = np.empty((512, NPAD), np.float16)
    for c in range(4):
        phase = 0.5 * np.pi if c < 2 else 0.0
        for g in range(4):
            k = 4 * (c % 2) + g + 1
            arg = k * xT + (phase + np.pi)
            red = np.mod(arg, 2 * np.pi) - np.pi
            R[128 * c + 32 * g: 128 * c + 32 * (g + 1), :] = red.astype(np.float16)
    return R


def _host_coeffs(fouriercoeffs):
    """[512, 64] bf16-as-uint16 view prep: row 128c + 32g + i -> fc[t, j, i, k-1]."""
    W = np.empty((512, OUT_F), np.float32)
    for c in range(4):
        t = 0 if c < 2 else 1
        for g in range(4):
            k = 4 * (c % 2) + g + 1
            # fc[t, :, :, k-1] is [OUT_F, IN_F] -> rows i, cols j
            W[128 * c + 32 * g: 128 * c + 32 * (g + 1), :] = fouriercoeffs[t, :, :, k - 1].T
    return _to_bf16(W)


def _to_bf16(a):
    """fp32 -> bf16 numpy array (ml_dtypes)."""
    import ml_dtypes
    return a.astype(np.float32).astype(ml_dtypes.bfloat16)


def _host_edges(src, dst):
    """Bucket edges into the padded slot schedule.

    Returns (schedule, idx16 [8,128,TOT/16], dstrel_bf16 [8,128,TOTG]).
    schedule: dict with Lg[49], Hg[49], supers list.
    """
    src = np.asarray(src, np.int64)
    dst = np.asarray(dst, np.int64)
    core = dst // NSH
    dst_local = dst - core * NSH
    blk = dst_local >> 7
    rel = dst_local & 127
    high = (src >= 32768).astype(np.int64)
    key = (core * NBLK + blk) * 2 + high
    nkey = N_CORES * NBLK * 2
    cnt = np.bincount(key, minlength=nkey)
    cnt_cbh = cnt.reshape(N_CORES, NBLK, 2)
    Lg = np.maximum(np.ceil(cnt_cbh[:, :, 0].max(axis=0) / 128).astype(np.int64), 0)
    Hg = np.maximum(np.ceil(cnt_cbh[:, :, 1].max(axis=0) / 128).astype(np.int64), 0)

    # slot layout per core: per super: [low slots of blocks | high slots]
    n_sup = math.ceil(NBLK / SUPER)
    base_low = np.zeros(NBLK, np.int64)
    base_high = np.zeros(NBLK, np.int64)
    supers = []
    pos = 0
    for s in range(n_sup):
        bs = list(range(SUPER * s, min(SUPER * (s + 1), NBLK)))
        sup = {"blocks": bs, "slot0": pos}
        for b in bs:
            base_low[b] = pos
            pos += Lg[b] * 128
        sup["low_end"] = pos
        for b in bs:
            base_high[b] = pos
            pos += Hg[b] * 128
        sup["end"] = pos
        supers.append(sup)
    TOT = pos
    TOTG = TOT // 128

    order = np.argsort(key, kind="stable")
    cnt_flat = cnt
    seg_start = np.concatenate([[0], np.cumsum(cnt_flat)[:-1]])
    key_s = key[order]
    rank_s = np.arange(N_EDGES, dtype=np.int64) - seg_start[key_s]
    blk_s = blk[order]
    high_s = high[order]
    core_s = core[order]
    src_s = src[order]
    rel_s = rel[order]
    slot_s = np.where(high_s == 0, base_low[blk_s], base_high[blk_s]) + rank_s

    idxval = (src_s - 32768 * high_s).astype(np.int16)
    idx16 = np.zeros((N_CORES, 16, TOT // 16), np.int16)
    idx16[core_s, slot_s % 16, slot_s // 16] = idxval
    idx16 = np.tile(idx16, (1, 8, 1))  # replicate to 128 partitions

    dstrel = np.full((N_CORES, 128, TOTG), 1000.0, np.float32)
    dstrel[core_s, slot_s % 128, slot_s // 128] = rel_s.astype(np.float32)
    dstrel16 = _to_bf16(dstrel)

    sched = {
        "Lg": tuple(int(v) for v in Lg),
        "Hg": tuple(int(v) for v in Hg),
        "supers": supers,
        "base_low": base_low,
        "base_high": base_high,
        "TOT": TOT,
        "TOTG": TOTG,
    }
    return sched, idx16, dstrel16


# ---------------------------------------------------------------- device program

def _build_program(sched, loop_n=1):
    import contextlib
    import concourse.bacc as bacc
    import concourse.mybir as mybir
    import concourse.tile as tile

    Lg, Hg = sched["Lg"], sched["Hg"]
    TOT, TOTG = sched["TOT"], sched["TOTG"]

    nc = bacc.Bacc("TRN2", target_bir_lowering=False, debug=False,
                   num_devices=N_CORES, num_swdge_queues=4)
    bf16 = mybir.dt.bfloat16
    fp16 = mybir.dt.float16
    fp32 = mybir.dt.float32
    i16 = mybir.dt.int16

    r2_in = nc.dram_tensor("r2", [512, NSH], fp16, kind="ExternalInput").ap()
    wp_in = nc.dram_tensor("wp", [512, OUT_F], bf16, kind="ExternalInput").ap()
    idx_in = nc.dram_tensor("idx16", [128, TOT // 16], i16, kind="ExternalInput").ap()
    dr_in = nc.dram_tensor("dstrel", [128, TOTG], bf16, kind="ExternalInput").ap()
    iota_in = nc.dram_tensor("iota", [128, 128], bf16, kind="ExternalInput").ap()
    h_out = nc.dram_tensor("h", [NSH, OUT_F], fp32, kind="ExternalOutput").ap()

    if DEBUG_PHASE == 2:
        mn_full = nc.dram_tensor("mn_in", [NPAD, MROW], bf16, kind="ExternalInput").ap()
        mn_slice = None
    else:
        mn_slice = nc.dram_tensor("mn_slice", [NSH, MROW], bf16).ap()
        mn_full = nc.dram_tensor("mn_full", [NPAD, MROW], bf16, addr_space="Shared").ap()

    with tile.TileContext(nc) as tc:
        with (
            tc.tile_pool(name="p1", bufs=2) as p1,
            tc.tile_pool(name="const", bufs=1) as constp,
            tc.tile_pool(name="psum", bufs=4, space="PSUM") as psp,
        ):
            run_p1 = DEBUG_PHASE in (0, 1, 3)
            run_p2 = DEBUG_PHASE in (0, 2)
            wp_t = constp.tile([128, 4, OUT_F], bf16)
            nc.sync.dma_start(out=wp_t[:], in_=wp_in.rearrange("(c p) j -> p c j", p=128))
            iota_t = constp.tile([128, 128], bf16)
            nc.sync.dma_start(out=iota_t[:], in_=iota_in[:])
            dr_t = constp.tile([128, TOTG], bf16)
            nc.sync.dma_start(out=dr_t[:], in_=dr_in[:])
            idx_t = constp.tile([128, TOT // 16], i16)
            nc.sync.dma_start(out=idx_t[:], in_=idx_in[:])
            h_sb = constp.tile([128, NBLK, OUT_F], fp32)

            def emit_phase1():
                tiles = [(s, min(1792, NSH - s)) for s in range(0, NSH, 1792)]
                for (t0, tn) in (tiles if run_p1 else []):
                    Fs = []
                    for c in range(4):
                        xr = p1.tile([128, tn], fp16, tag="xr")
                        nc.sync.dma_start(out=xr[:], in_=r2_in[128 * c:128 * (c + 1), t0:t0 + tn])
                        Fc = p1.tile([128, tn], bf16, tag=f"F{c}")
                        nc.scalar.activation(Fc[:], xr[:], mybir.ActivationFunctionType.Sin)
                        Fs.append(Fc)
                    stag = p1.tile([128, tn // 128, MROW], bf16, tag="stag")
                    nc.scalar.memzero(stag[:])
                    for sub in range(tn // 128):
                        ps = psp.tile([128, OUT_F], fp32, tag="ps1")
                        for c in range(4):
                            nc.tensor.matmul(
                                out=ps[:],
                                lhsT=Fs[c][:, sub * 128:(sub + 1) * 128],
                                rhs=wp_t[:, c, :],
                                start=(c == 0), stop=(c == 3),
                            )
                        nc.vector.tensor_copy(out=stag[:, sub, 0:OUT_F], in_=ps[:])
                    nc.sync.dma_start(
                        out=mn_slice[t0:t0 + tn, :].rearrange("(s p) d -> p s d", p=128),
                        in_=stag[:],
                    )

            def emit_cc():
                if run_p1 and DEBUG_PHASE not in (2, 3):
                    nc.gpsimd.collective_compute(
                        "AllGather", mybir.AluOpType.bypass,
                        replica_groups=[list(range(N_CORES))],
                        ins=[mn_slice[:]], outs=[mn_full[:]],
                    )

            def emit_phase2():
                if not run_p2 or SKIP_MM:
                    nc.scalar.memzero(h_sb[:])
                # Greedy-balance gather desc-gen across the 4 SWDGE Q7 pairs;
                # chunk spans so all queues work within the in-flight window.
                qload = [0, 0, 0, 0]

                def q_for(n):
                    q = min(range(4), key=lambda i: qload[i])
                    qload[q] += n
                    return q

                CHUNK = 4096

                def emit_gather(me, slot0, span_a, span_b, base_high):
                    for a in range(span_a, span_b, CHUNK):
                        b = min(a + CHUNK, span_b)
                        nc.gpsimd.dma_gather(
                            out_ap=me[:, (a - slot0) // 128:(b - slot0) // 128, :],
                            in_ap=mn_full[32768:, :] if base_high else mn_full[:],
                            idxs_ap=idx_t[:, a // 16: b // 16],
                            num_idxs=b - a, num_idxs_reg=b - a, elem_size=MROW,
                            single_packet=False, queue_num=q_for(b - a),
                        )
                for sup in (sched["supers"] if run_p2 else []):
                    bs = sup["blocks"]
                    slot0, low_end, end = sup["slot0"], sup["low_end"], sup["end"]
                    gsup = (end - slot0) // 128
                    if gsup == 0:
                        for b in bs:
                            nc.scalar.memzero(h_sb[:, b, :])
                        continue
                    g0 = slot0 // 128
                    me = p1.tile([128, gsup, MROW], bf16, tag="me", bufs=3)
                    n_low = low_end - slot0
                    n_high = end - low_end
                    if SKIP_GATHER:
                        n_low = n_high = 0
                    if n_low > 0:
                        emit_gather(me, slot0, slot0, low_end, base_high=False)
                    if n_high > 0:
                        emit_gather(me, slot0, low_end, end, base_high=True)
                    if SKIP_MM:
                        continue
                    oh = p1.tile([128, gsup, 128], bf16, tag="oh")
                    nc.vector.tensor_tensor(
                        out=oh[:],
                        in0=dr_t[:, g0:g0 + gsup].unsqueeze(2).to_broadcast([128, gsup, 128]),
                        in1=iota_t[:].unsqueeze(1).to_broadcast([128, gsup, 128]),
                        op=mybir.AluOpType.is_equal,
                    )
                    for b in bs:
                        cols = (
                            [(sched["base_low"][b] - slot0) // 128 + j for j in range(Lg[b])]
                            + [(sched["base_high"][b] - slot0) // 128 + j for j in range(Hg[b])]
                        )
                        if not cols:
                            nc.scalar.memzero(h_sb[:, b, :])
                            continue
                        ps = psp.tile([128, OUT_F], fp32, tag="ps2")
                        for gi, col in enumerate(cols):
                            nc.tensor.matmul(
                                out=ps[:],
                                lhsT=oh[:, col, :],
                                rhs=me[:, col, 0:OUT_F],
                                start=(gi == 0), stop=(gi == len(cols) - 1),
                            )
                        nc.vector.tensor_copy(out=h_sb[:, b, :], in_=ps[:])

            if loop_n == 1:
                emit_phase1()
                emit_cc()
                emit_phase2()
            else:
                # timing mode: collective once (on whatever mn_slice holds),
                # then loop phase1+phase2 without the collective.
                emit_cc()
                with tc.For_i(0, loop_n, 1):
                    emit_phase1()
                    emit_phase2()

            nc.sync.dma_start(
                out=h_out[:].rearrange("(b p) j -> p b j", p=128),
                in_=h_sb[:],
            )

    nc.compile()
    return nc


def _build_cc_probe(nreps):
    """Program issuing `nreps` back-to-back AllGathers (for collective timing)."""
    import concourse.bacc as bacc
    import concourse.mybir as mybir
    import concourse.tile as tile

    nc = bacc.Bacc("TRN2", target_bir_lowering=False, debug=False,
                   num_devices=N_CORES)
    bf16 = mybir.dt.bfloat16
    dummy_in = nc.dram_tensor("d", [128, 16], mybir.dt.float32, kind="ExternalInput").ap()
    dummy_out = nc.dram_tensor("o", [128, 16], mybir.dt.float32, kind="ExternalOutput").ap()
    mn_slice = nc.dram_tensor("mn_slice", [NSH, MROW], bf16).ap()
    mn_full = nc.dram_tensor("mn_full", [NPAD, MROW], bf16, addr_space="Shared").ap()
    with tile.TileContext(nc) as tc:
        with tc.tile_pool(name="s", bufs=1) as pool:
            t = pool.tile([128, 16], mybir.dt.float32)
            nc.sync.dma_start(out=t[:], in_=dummy_in[:])
            for _ in range(nreps):
                nc.gpsimd.collective_compute(
                    "AllGather", mybir.AluOpType.bypass,
                    replica_groups=[list(range(N_CORES))],
                    ins=[mn_slice[:]], outs=[mn_full[:]],
                )
            nc.sync.dma_start(out=dummy_out[:], in_=t[:])
    nc.compile()
    return nc


# ---------------------------------------------------------------- runner

def _make_runner(nc):
    """Cached-jit SPMD runner (replicates bass2jax.run_bass_via_pjrt)."""
    import jax
    import jax.numpy as jnp
    import numpy as _np
    from jax.sharding import Mesh, PartitionSpec
    from jax.experimental.shard_map import shard_map
    import concourse.mybir as mybir
    from concourse import bass2jax

    bass2jax.install_neuronx_cc_hook()

    partition_name = nc.partition_id_tensor.name if nc.partition_id_tensor else None
    in_names, out_names, out_avals = [], [], []
    for alloc in nc.m.functions[0].allocations:
        if not isinstance(alloc, mybir.MemoryLocationSet):
            continue
        name = alloc.memorylocations[0].name
        if alloc.kind == "ExternalInput":
            if name != partition_name:
                in_names.append(name)
        elif alloc.kind == "ExternalOutput":
            shape = tuple(alloc.tensor_shape)
            dtype = mybir.dt.np(alloc.dtype)
            out_names.append(name)
            out_avals.append(jax.core.ShapedArray(shape, dtype))
    n_params = len(in_names)
    all_in_names = in_names + out_names
    if partition_name is not None:
        all_in_names = all_in_names + [partition_name]

    def _body(*args):
        operands = list(args)
        if partition_name is not None:
            operands.append(bass2jax.partition_id_tensor())
        outs = bass2jax._bass_exec_p.bind(
            *operands,
            out_avals=tuple(out_avals),
            in_names=tuple(all_in_names),
            out_names=tuple(out_names),
            lowering_input_output_aliases=(),
            sim_require_finite=False,
            sim_require_nnan=False,
            nc=nc,
        )
        return tuple(outs)

    devices = jax.devices()[:N_CORES]
    mesh = Mesh(_np.asarray(devices), ("core",))
    in_specs = (PartitionSpec("core"),) * (n_params + len(out_names))
    out_specs = (PartitionSpec("core"),) * len(out_names)
    sharded = jax.jit(shard_map(_body, mesh=mesh, in_specs=in_specs,
                                out_specs=out_specs, check_rep=False))

    def make_multi(nreps):
        def _multi(*args):
            outs = None
            for _ in range(nreps):
                outs = _body(*args)
            return outs
        return jax.jit(shard_map(_multi, mesh=mesh, in_specs=in_specs,
                                 out_specs=out_specs, check_rep=False))

    def put(in_maps):
        concat = [
            _np.concatenate([_np.asarray(in_maps[c][n]) for c in range(N_CORES)], axis=0)
            for n in in_names
        ]
        concat += [
            _np.zeros((N_CORES * a.shape[0], *a.shape[1:]), a.dtype) for a in out_avals
        ]
        return [jax.device_put(a) for a in concat]

    def dispatch(dev_inputs):
        outs = sharded(*dev_inputs)
        for o in outs:
            o.block_until_ready()
        return outs

    def fetch(outs):
        return {
            name: _np.asarray(outs[i]).reshape(N_CORES, *out_avals[i].shape)
            for i, name in enumerate(out_names)
        }

    def run(dev_inputs):
        return fetch(dispatch(dev_inputs))

    run.dispatch = dispatch
    run.fetch = fetch
    run.make_multi = make_multi
    return put, run


# ---------------------------------------------------------------- entry

def prepare(x, src, dst, fouriercoeffs, bias):
    """Build inputs + compiled program. Returns (runner_put, runner_run, in_maps, bias)."""
    R2 = _host_planes(np.asarray(x, np.float32))
    Wp = _host_coeffs(np.asarray(fouriercoeffs, np.float32))
    sched, idx16, dstrel16 = _host_edges(src, dst)
    iota = _to_bf16(np.tile(np.arange(128, dtype=np.float32)[None, :], (128, 1)))

    key = (sched["Lg"], sched["Hg"], LOOP_N)
    if key not in _COMPILED:
        nc = _build_program(sched, loop_n=LOOP_N)
        _COMPILED[key] = (nc, _make_runner(nc))
    nc, (put, run) = _COMPILED[key]

    in_maps = []
    for c in range(N_CORES):
        in_maps.append({
            "r2": R2[:, c * NSH:(c + 1) * NSH].copy(),
            "wp": Wp,
            "idx16": idx16[c],
            "dstrel": dstrel16[c],
            "iota": iota,
        })
    return put, run, in_maps


def kernel(x, src, dst, fouriercoeffs, bias):
    put, run, in_maps = prepare(x, src, dst, fouriercoeffs, bias)
    dev = put(in_maps)
    outs = run(dev)
    h = outs["h"].reshape(NPAD, OUT_F)[:N_NODES]
    return (h + np.asarray(bias, np.float32)[None, :]).astype(np.float32)



# revision 9
# speedup vs baseline: 2.8821x; 1.4236x over previous
"""NaiveFourierKANLayer GNN message passing on 8 Trainium2 NeuronCores.

Strategy:
  m_node[n, j] = sum_{i,k} cos(k x[n,i]) Wc[j,i,k] + sin(k x[n,i]) Ws[j,i,k]
  h[d, j]     = sum_{e: dst[e]=d} m_node[src[e], j]  (+ bias)

- Host: range-reduce the 16 (trig, k) argument planes into [-pi, pi) fp16
  (Sin activation spline is only valid there), bucket edges by
  (dst-block-of-128, src<32768) into a padded slot schedule shared by all
  cores (dst-range edge sharding: core c owns dst in [c*6272, (c+1)*6272)).
- Phase 1 (node-sharded): each core computes m_node for its 6272 nodes:
  ACT Sin -> bf16 Fourier features, PE matmul vs coeffs -> PSUM fp32,
  written to a bf16 [6272, 128] slice (64 msg + 64 zero pad for 256B rows).
- AllGather the slices -> full m_node [50176, 128] bf16 on every core.
- Phase 2 (edge-sharded by dst range): bulk dma_gather of m_node rows per
  edge slot (two passes: src<32768 and src>=32768, int16 idx limit), build
  one-hot(dst_rel) matrices with a DVE is_equal, and scatter-reduce with
  the tensor engine: psum[dst, j] += onehot^T @ m_edge, per 128-dst block.
- Output h slices are concatenated on host; bias added on host.
"""
import math
import numpy as np

N_NODES = 50000
N_EDGES = 800000
IN_F = 32
OUT_F = 64
GRID = 8
N_CORES = 8
NSH = 6272                    # nodes per core (49 * 128)
NPAD = NSH * N_CORES          # 50176
NBLK = NSH // 128             # 49 dst blocks per core
SUPER = 4                     # dst blocks per gather super-group
MROW = 128                    # m_node row width (64 msg + 64 pad) -> 256B bf16

_COMPILED = {}
DEBUG_PHASE = 0
LOOP_N = 1
SKIP_GATHER = False
SKIP_MM = False  # 0=full, 1=phase1+allgather only (h=0), 2=phase2 only (mn as input)


# ---------------------------------------------------------------- host prep

def _host_planes(x):
    """[512, NPAD] fp16 of range-reduced sin args: row 128c + 32g + i."""
    xT = np.zeros((IN_F, NPAD), np.float32)
    xT[:, :N_NODES] = x.T
    R = np.empty((512, NPAD), np.float16)
    for c in range(4):
        phase = 0.5 * np.pi if c < 2 else 0.0
        for g in range(4):
            k = 4 * (c % 2) + g + 1
            arg = k * xT + (phase + np.pi)
            red = np.mod(arg, 2 * np.pi) - np.pi
            R[128 * c + 32 * g: 128 * c + 32 * (g + 1), :] = red.astype(np.float16)
    return R


def _host_coeffs(fouriercoeffs):
    """[512, 64] bf16-as-uint16 view prep: row 128c + 32g + i -> fc[t, j, i, k-1]."""
    W = np.empty((512, OUT_F), np.float32)
    for c in range(4):
        t = 0 if c < 2 else 1
        for g in range(4):
            k = 4 * (c % 2) + g + 1
            # fc[t, :, :, k-1] is [OUT_F, IN_F] -> rows i, cols j
            W[128 * c + 32 * g: 128 * c + 32 * (g + 1), :] = fouriercoeffs[t, :, :, k - 1].T
    return _to_bf16(W)


def _to_bf16(a):
    """fp32 -> bf16 numpy array (ml_dtypes)."""
    import ml_dtypes
    return a.astype(np.float32).astype(ml_dtypes.bfloat16)


def _host_edges(src, dst):
    """Bucket edges into the padded slot schedule.

    Returns (schedule, idx16 [8,128,TOT/16], dstrel_bf16 [8,128,TOTG]).
    schedule: dict with Lg[49], Hg[49], supers list.
    """
    src = np.asarray(src, np.int64)
    dst = np.asarray(dst, np.int64)
    core = dst // NSH
    dst_local = dst - core * NSH
    blk = dst_local >> 7
    rel = dst_local & 127
    high = (src >= 32768).astype(np.int64)
    key = (core * NBLK + blk) * 2 + high
    nkey = N_CORES * NBLK * 2
    cnt = np.bincount(key, minlength=nkey)
    cnt_cbh = cnt.reshape(N_CORES, NBLK, 2)
    Lg = np.maximum(np.ceil(cnt_cbh[:, :, 0].max(axis=0) / 128).astype(np.int64), 0)
    Hg = np.maximum(np.ceil(cnt_cbh[:, :, 1].max(axis=0) / 128).astype(np.int64), 0)

    # slot layout per core: per super: [low slots of blocks | high slots]
    n_sup = math.ceil(NBLK / SUPER)
    base_low = np.zeros(NBLK, np.int64)
    base_high = np.zeros(NBLK, np.int64)
    supers = []
    pos = 0
    for s in range(n_sup):
        bs = list(range(SUPER * s, min(SUPER * (s + 1), NBLK)))
        sup = {"blocks": bs, "slot0": pos}
        for b in bs:
            base_low[b] = pos
            pos += Lg[b] * 128
        sup["low_end"] = pos
        for b in bs:
            base_high[b] = pos
            pos += Hg[b] * 128
        sup["end"] = pos
        supers.append(sup)
    TOT = pos
    TOTG = TOT // 128

    order = np.argsort(key, kind="stable")
    cnt_flat = cnt
    seg_start = np.concatenate([[0], np.cumsum(cnt_flat)[:-1]])
    key_s = key[order]
    rank_s = np.arange(N_EDGES, dtype=np.int64) - seg_start[key_s]
    blk_s = blk[order]
    high_s = high[order]
    core_s = core[order]
    src_s = src[order]
    rel_s = rel[order]
    slot_s = np.where(high_s == 0, base_low[blk_s], base_high[blk_s]) + rank_s

    idxval = (src_s - 32768 * high_s).astype(np.int16)
    idx16 = np.zeros((N_CORES, 16, TOT // 16), np.int16)
    idx16[core_s, slot_s % 16, slot_s // 16] = idxval
    idx16 = np.tile(idx16, (1, 8, 1))  # replicate to 128 partitions

    dstrel = np.full((N_CORES, 128, TOTG), 1000.0, np.float32)
    dstrel[core_s, slot_s % 128, slot_s // 128] = rel_s.astype(np.float32)
    dstrel16 = _to_bf16(dstrel)

    sched = {
        "Lg": tuple(int(v) for v in Lg),
        "Hg": tuple(int(v) for v in Hg),
        "supers": supers,
        "base_low": base_low,
        "base_high": base_high,
        "TOT": TOT,
        "TOTG": TOTG,
    }
    return sched, idx16, dstrel16


# ---------------------------------------------------------------- device program

def _build_program(sched, loop_n=1):
    import contextlib
    import concourse.bacc as bacc
    import concourse.mybir as mybir
    import concourse.tile as tile

    Lg, Hg = sched["Lg"], sched["Hg"]
    TOT, TOTG = sched["TOT"], sched["TOTG"]

    nc = bacc.Bacc("TRN2", target_bir_lowering=False, debug=False,
                   num_devices=N_CORES, num_swdge_queues=4)
    bf16 = mybir.dt.bfloat16
    fp16 = mybir.dt.float16
    fp32 = mybir.dt.float32
    i16 = mybir.dt.int16

    r2_in = nc.dram_tensor("r2", [512, NSH], fp16, kind="ExternalInput").ap()
    wp_in = nc.dram_tensor("wp", [512, OUT_F], bf16, kind="ExternalInput").ap()
    idx_in = nc.dram_tensor("idx16", [128, TOT // 16], i16, kind="ExternalInput").ap()
    dr_in = nc.dram_tensor("dstrel", [128, TOTG], bf16, kind="ExternalInput").ap()
    iota_in = nc.dram_tensor("iota", [128, 128], bf16, kind="ExternalInput").ap()
    h_out = nc.dram_tensor("h", [NSH, OUT_F], fp32, kind="ExternalOutput").ap()

    if DEBUG_PHASE == 2:
        mn_full = nc.dram_tensor("mn_in", [NPAD, MROW], bf16, kind="ExternalInput").ap()
        mn_slice = None
    else:
        mn_slice = nc.dram_tensor("mn_slice", [NSH, MROW], bf16).ap()
        mn_full = nc.dram_tensor("mn_full", [NPAD, MROW], bf16, addr_space="Shared").ap()

    with tile.TileContext(nc) as tc:
        with (
            tc.tile_pool(name="p1", bufs=2) as p1,
            tc.tile_pool(name="const", bufs=1) as constp,
            tc.tile_pool(name="psum", bufs=4, space="PSUM") as psp,
        ):
            run_p1 = DEBUG_PHASE in (0, 1, 3)
            run_p2 = DEBUG_PHASE in (0, 2)
            wp_t = constp.tile([128, 4, OUT_F], bf16)
            nc.sync.dma_start(out=wp_t[:], in_=wp_in.rearrange("(c p) j -> p c j", p=128))
            iota_t = constp.tile([128, 128], bf16)
            nc.sync.dma_start(out=iota_t[:], in_=iota_in[:])
            dr_t = constp.tile([128, TOTG], bf16)
            nc.sync.dma_start(out=dr_t[:], in_=dr_in[:])
            idx_t = constp.tile([128, TOT // 16], i16)
            nc.sync.dma_start(out=idx_t[:], in_=idx_in[:])
            h_sb = constp.tile([128, NBLK, OUT_F], fp32)

            def emit_phase1():
                tiles = [(s, min(1792, NSH - s)) for s in range(0, NSH, 1792)]
                for (t0, tn) in (tiles if run_p1 else []):
                    Fs = []
                    for c in range(4):
                        xr = p1.tile([128, tn], fp16, tag="xr")
                        nc.sync.dma_start(out=xr[:], in_=r2_in[128 * c:128 * (c + 1), t0:t0 + tn])
                        Fc = p1.tile([128, tn], bf16, tag=f"F{c}")
                        nc.scalar.activation(Fc[:], xr[:], mybir.ActivationFunctionType.Sin)
                        Fs.append(Fc)
                    stag = p1.tile([128, tn // 128, MROW], bf16, tag="stag")
                    nc.scalar.memzero(stag[:])
                    for sub in range(tn // 128):
                        ps = psp.tile([128, OUT_F], fp32, tag="ps1")
                        for c in range(4):
                            nc.tensor.matmul(
                                out=ps[:],
                                lhsT=Fs[c][:, sub * 128:(sub + 1) * 128],
                                rhs=wp_t[:, c, :],
                                start=(c == 0), stop=(c == 3),
                            )
                        nc.vector.tensor_copy(out=stag[:, sub, 0:OUT_F], in_=ps[:])
                    nc.sync.dma_start(
                        out=mn_slice[t0:t0 + tn, :].rearrange("(s p) d -> p s d", p=128),
                        in_=stag[:],
                    )

            def emit_cc():
                if run_p1 and DEBUG_PHASE not in (2, 3):
                    nc.gpsimd.collective_compute(
                        "AllGather", mybir.AluOpType.bypass,
                        replica_groups=[list(range(N_CORES))],
                        ins=[mn_slice[:]], outs=[mn_full[:]],
                    )

            def emit_phase2():
                if not run_p2 or SKIP_MM:
                    nc.scalar.memzero(h_sb[:])
                # Spread gather desc-gen across the 4 SWDGE Q7 pairs: strict
                # round-robin queue order (the Pool dispatch window is only
                # ~4 deep, so two consecutive chunks on one queue stall it)
                # with near-equal chunk sizes for balance.
                qn = [0]
                CHUNK = 4096

                def emit_gather(me, slot0, span_a, span_b, base_high):
                    n = span_b - span_a
                    pieces = max(1, -(-n // CHUNK))
                    step = -(-(n // 128) // pieces) * 128
                    for a in range(span_a, span_b, step):
                        b = min(a + step, span_b)
                        q = qn[0]
                        qn[0] = (q + 1) % 4
                        nc.gpsimd.dma_gather(
                            out_ap=me[:, (a - slot0) // 128:(b - slot0) // 128, :],
                            in_ap=mn_full[32768:, :] if base_high else mn_full[:],
                            idxs_ap=idx_t[:, a // 16: b // 16],
                            num_idxs=b - a, num_idxs_reg=b - a, elem_size=MROW,
                            single_packet=False, queue_num=q,
                        )
                for sup in (sched["supers"] if run_p2 else []):
                    bs = sup["blocks"]
                    slot0, low_end, end = sup["slot0"], sup["low_end"], sup["end"]
                    gsup = (end - slot0) // 128
                    if gsup == 0:
                        for b in bs:
                            nc.scalar.memzero(h_sb[:, b, :])
                        continue
                    g0 = slot0 // 128
                    me = p1.tile([128, gsup, MROW], bf16, tag="me", bufs=3)
                    n_low = low_end - slot0
                    n_high = end - low_end
                    if SKIP_GATHER:
                        n_low = n_high = 0
                    if n_low > 0:
                        emit_gather(me, slot0, slot0, low_end, base_high=False)
                    if n_high > 0:
                        emit_gather(me, slot0, low_end, end, base_high=True)
                    if SKIP_MM:
                        continue
                    oh = p1.tile([128, gsup, 128], bf16, tag="oh")
                    nc.vector.tensor_tensor(
                        out=oh[:],
                        in0=dr_t[:, g0:g0 + gsup].unsqueeze(2).to_broadcast([128, gsup, 128]),
                        in1=iota_t[:].unsqueeze(1).to_broadcast([128, gsup, 128]),
                        op=mybir.AluOpType.is_equal,
                    )
                    for b in bs:
                        cols = (
                            [(sched["base_low"][b] - slot0) // 128 + j for j in range(Lg[b])]
                            + [(sched["base_high"][b] - slot0) // 128 + j for j in range(Hg[b])]
                        )
                        if not cols:
                            nc.scalar.memzero(h_sb[:, b, :])
                            continue
                        ps = psp.tile([128, OUT_F], fp32, tag="ps2")
                        for gi, col in enumerate(cols):
                            nc.tensor.matmul(
                                out=ps[:],
                                lhsT=oh[:, col, :],
                                rhs=me[:, col, 0:OUT_F],
                                start=(gi == 0), stop=(gi == len(cols) - 1),
                            )
                        nc.vector.tensor_copy(out=h_sb[:, b, :], in_=ps[:])

            if loop_n == 1:
                emit_phase1()
                emit_cc()
                emit_phase2()
            else:
                # timing mode: collective once (on whatever mn_slice holds),
                # then loop phase1+phase2 without the collective.
                emit_cc()
                with tc.For_i(0, loop_n, 1):
                    emit_phase1()
                    emit_phase2()

            nc.sync.dma_start(
                out=h_out[:].rearrange("(b p) j -> p b j", p=128),
                in_=h_sb[:],
            )

    nc.compile()
    return nc


def _build_cc_probe(nreps):
    """Program issuing `nreps` back-to-back AllGathers (for collective timing)."""
    import concourse.bacc as bacc
    import concourse.mybir as mybir
    import concourse.tile as tile

    nc = bacc.Bacc("TRN2", target_bir_lowering=False, debug=False,
                   num_devices=N_CORES)
    bf16 = mybir.dt.bfloat16
    dummy_in = nc.dram_tensor("d", [128, 16], mybir.dt.float32, kind="ExternalInput").ap()
    dummy_out = nc.dram_tensor("o", [128, 16], mybir.dt.float32, kind="ExternalOutput").ap()
    mn_slice = nc.dram_tensor("mn_slice", [NSH, MROW], bf16).ap()
    mn_full = nc.dram_tensor("mn_full", [NPAD, MROW], bf16, addr_space="Shared").ap()
    with tile.TileContext(nc) as tc:
        with tc.tile_pool(name="s", bufs=1) as pool:
            t = pool.tile([128, 16], mybir.dt.float32)
            nc.sync.dma_start(out=t[:], in_=dummy_in[:])
            for _ in range(nreps):
                nc.gpsimd.collective_compute(
                    "AllGather", mybir.AluOpType.bypass,
                    replica_groups=[list(range(N_CORES))],
                    ins=[mn_slice[:]], outs=[mn_full[:]],
                )
            nc.sync.dma_start(out=dummy_out[:], in_=t[:])
    nc.compile()
    return nc


# ---------------------------------------------------------------- runner

def _make_runner(nc):
    """Cached-jit SPMD runner (replicates bass2jax.run_bass_via_pjrt)."""
    import jax
    import jax.numpy as jnp
    import numpy as _np
    from jax.sharding import Mesh, PartitionSpec
    from jax.experimental.shard_map import shard_map
    import concourse.mybir as mybir
    from concourse import bass2jax

    bass2jax.install_neuronx_cc_hook()

    partition_name = nc.partition_id_tensor.name if nc.partition_id_tensor else None
    in_names, out_names, out_avals = [], [], []
    for alloc in nc.m.functions[0].allocations:
        if not isinstance(alloc, mybir.MemoryLocationSet):
            continue
        name = alloc.memorylocations[0].name
        if alloc.kind == "ExternalInput":
            if name != partition_name:
                in_names.append(name)
        elif alloc.kind == "ExternalOutput":
            shape = tuple(alloc.tensor_shape)
            dtype = mybir.dt.np(alloc.dtype)
            out_names.append(name)
            out_avals.append(jax.core.ShapedArray(shape, dtype))
    n_params = len(in_names)
    all_in_names = in_names + out_names
    if partition_name is not None:
        all_in_names = all_in_names + [partition_name]

    def _body(*args):
        operands = list(args)
        if partition_name is not None:
            operands.append(bass2jax.partition_id_tensor())
        outs = bass2jax._bass_exec_p.bind(
            *operands,
            out_avals=tuple(out_avals),
            in_names=tuple(all_in_names),
            out_names=tuple(out_names),
            lowering_input_output_aliases=(),
            sim_require_finite=False,
            sim_require_nnan=False,
            nc=nc,
        )
        return tuple(outs)

    devices = jax.devices()[:N_CORES]
    mesh = Mesh(_np.asarray(devices), ("core",))
    in_specs = (PartitionSpec("core"),) * (n_params + len(out_names))
    out_specs = (PartitionSpec("core"),) * len(out_names)
    sharded = jax.jit(shard_map(_body, mesh=mesh, in_specs=in_specs,
                                out_specs=out_specs, check_rep=False))

    def make_multi(nreps):
        def _multi(*args):
            outs = None
            for _ in range(nreps):
                outs = _body(*args)
            return outs
        return jax.jit(shard_map(_multi, mesh=mesh, in_specs=in_specs,
                                 out_specs=out_specs, check_rep=False))

    def put(in_maps):
        concat = [
            _np.concatenate([_np.asarray(in_maps[c][n]) for c in range(N_CORES)], axis=0)
            for n in in_names
        ]
        concat += [
            _np.zeros((N_CORES * a.shape[0], *a.shape[1:]), a.dtype) for a in out_avals
        ]
        return [jax.device_put(a) for a in concat]

    def dispatch(dev_inputs):
        outs = sharded(*dev_inputs)
        for o in outs:
            o.block_until_ready()
        return outs

    def fetch(outs):
        return {
            name: _np.asarray(outs[i]).reshape(N_CORES, *out_avals[i].shape)
            for i, name in enumerate(out_names)
        }

    def run(dev_inputs):
        return fetch(dispatch(dev_inputs))

    run.dispatch = dispatch
    run.fetch = fetch
    run.make_multi = make_multi
    return put, run


# ---------------------------------------------------------------- entry

def prepare(x, src, dst, fouriercoeffs, bias):
    """Build inputs + compiled program. Returns (runner_put, runner_run, in_maps, bias)."""
    R2 = _host_planes(np.asarray(x, np.float32))
    Wp = _host_coeffs(np.asarray(fouriercoeffs, np.float32))
    sched, idx16, dstrel16 = _host_edges(src, dst)
    iota = _to_bf16(np.tile(np.arange(128, dtype=np.float32)[None, :], (128, 1)))

    key = (sched["Lg"], sched["Hg"], LOOP_N)
    if key not in _COMPILED:
        nc = _build_program(sched, loop_n=LOOP_N)
        _COMPILED[key] = (nc, _make_runner(nc))
    nc, (put, run) = _COMPILED[key]

    in_maps = []
    for c in range(N_CORES):
        in_maps.append({
            "r2": R2[:, c * NSH:(c + 1) * NSH].copy(),
            "wp": Wp,
            "idx16": idx16[c],
            "dstrel": dstrel16[c],
            "iota": iota,
        })
    return put, run, in_maps


def kernel(x, src, dst, fouriercoeffs, bias):
    put, run, in_maps = prepare(x, src, dst, fouriercoeffs, bias)
    dev = put(in_maps)
    outs = run(dev)
    h = outs["h"].reshape(NPAD, OUT_F)[:N_NODES]
    return (h + np.asarray(bias, np.float32)[None, :]).astype(np.float32)

